# revision 12
# baseline (speedup 1.0000x reference)
"""GraphormerPooling kernel for 8 TRN2 NeuronCores.

The reference module has two structural dead-code properties (verified exactly
against the jax reference on perturbed inputs, rel err ~5e-7):
  1. The layer loop feeds the SAME input x0 to every layer and overwrites
     layer_out, so only the last layer (l=3) contributes to the output.
  2. The output is CLS pooling (out[:, 0]) and every post-attention op is
     per-position, so only the CLS row of layer 3 is needed. The CLS query
     attends over all S=257 keys, which still requires LN over all of x.

With K = h@Wk + bk and V = h@Wv + bv folded algebraically:
  scores[b,h,s] = (h~[b,s,:]Â·a'[h] + kappa[h]) / 8 + mask[b,s]
      a'[h,d] = g[d] * sum_{d' in head h} Wk[d,d'] q0[d']   (A-matrix)
  u[b,h,:]  = sum_s p0[b,h,s] h~[b,s,:]  (then *g, + sum(p0)*b_ln rank-1)
  o0[b, head h slice] = u[b,h,:] @ Wv[:, head slice] + sum(p0)*bv[slice]
so no (B*S,D)x(D,D) projections are ever materialized. Sharding: data-parallel
over batch, 4 batches per core; all weights replicated (layer-3 slices only).
"""

import os
import sys
import numpy as np
from contextlib import ExitStack

for _p in ("/opt/trn_rl_repo", "/root/.axon_site/_ro/trn_rl_repo"):
    if os.path.isdir(_p) and _p not in sys.path:
        sys.path.insert(0, _p)

import concourse.bass as bass
import concourse.bacc as bacc
import concourse.tile as tile
from concourse import mybir
from concourse.bass_utils import run_bass_kernel_spmd
from concourse.masks import make_identity

F32 = mybir.dt.float32
B, N, D, H = 32, 256, 512, 8
S = N + 1
DK = D // H          # 64
DFF = 4 * D          # 2048
LAYER = 3
NCORES = 8
BPC = B // NCORES    # 4 batches per core
ROWS = BPC * N       # 1024 rows of x per core
NEG = -1.0e30
EPS = 1e-5

# vecs packing rows (width 512)
R_CLS, R_G1, R_B1, R_BQ, R_BK, R_BV, R_BO = 0, 1, 2, 3, 4, 5, 6
R_G2, R_B2, R_GF, R_BF, R_BF2 = 7, 8, 9, 10, 11
R_BF1 = 12           # rows 12..15
R_BIAS0 = 16         # rows 16..23  (attn_bias[:, 0, :], 257 cols)
R_MASK = 24          # rows 24..27  (per-core additive key mask, 257 cols)
NV = 28

_CACHE = {}
LAST_RESULTS = None  # test harness reads exec_time_ns / profile from here


def _build_nc():
    nc = bacc.Bacc()
    x_d = nc.declare_dram_parameter("x", [ROWS, D], F32, isOutput=False)
    wq_d = nc.declare_dram_parameter("wq", [D, D], F32, isOutput=False)
    wkt_d = nc.declare_dram_parameter("wkt", [D, D], F32, isOutput=False)
    wv_d = nc.declare_dram_parameter("wv", [D, D], F32, isOutput=False)
    wo_d = nc.declare_dram_parameter("wo", [D, D], F32, isOutput=False)
    wf1_d = nc.declare_dram_parameter("wf1", [D, DFF], F32, isOutput=False)
    wf2_d = nc.declare_dram_parameter("wf2", [DFF, D], F32, isOutput=False)
    vecs_d = nc.declare_dram_parameter("vecs", [NV, 512], F32, isOutput=False)
    out_d = nc.declare_dram_parameter("out", [BPC, D], F32, isOutput=True)

    with tile.TileContext(nc) as tc:
        with ExitStack() as ctx:
            _emit(ctx, tc, nc, x_d, wq_d, wkt_d, wv_d, wo_d, wf1_d, wf2_d,
                  vecs_d, out_d)
    nc.compile()
    return nc


def _emit(ctx, tc, nc, x_d, wq_d, wkt_d, wv_d, wo_d, wf1_d, wf2_d, vecs_d,
          out_d):
    const = ctx.enter_context(tc.tile_pool(name="const", bufs=1))
    work = ctx.enter_context(tc.tile_pool(name="work", bufs=2))
    small = ctx.enter_context(tc.tile_pool(name="small", bufs=2))
    tp_ps = ctx.enter_context(tc.tile_pool(name="tp_ps", bufs=2, space="PSUM"))
    mm_ps = ctx.enter_context(tc.tile_pool(name="mm_ps", bufs=2, space="PSUM"))
    sc_ps = ctx.enter_context(tc.tile_pool(name="sc_ps", bufs=2, space="PSUM"))
    ff_ps = ctx.enter_context(tc.tile_pool(name="ff_ps", bufs=1, space="PSUM"))

    # ---- DMAs, roughly in consumption order (x + vecs first) ----
    # Compute engines can only address SBUF at base partition 0/32/64/96, so
    # the packed parameter rows are DMA'd into partition-0-based tiles.
    V1 = const.tile([1, 12, 512], F32)          # rows 0..11, one per free slot
    for r in range(12):
        nc.sync.dma_start(out=V1[0:1, r, :], in_=vecs_d[r:r + 1, :])
    BF1 = const.tile([4, 512], F32)
    nc.sync.dma_start(out=BF1, in_=vecs_d[R_BF1:R_BF1 + 4, :])
    BIAS0 = const.tile([H, 512], F32)
    nc.sync.dma_start(out=BIAS0, in_=vecs_d[R_BIAS0:R_BIAS0 + H, :])
    maskr = const.tile([1, BPC, S], F32)
    for b in range(BPC):
        nc.sync.dma_start(out=maskr[0:1, b, :],
                          in_=vecs_d[R_MASK + b:R_MASK + b + 1, 0:S])

    hx = const.tile([128, 8, D], F32)          # raw x, row = b*256+n
    for c in range(8):
        nc.sync.dma_start(out=hx[:, c, :], in_=x_d[c * 128:(c + 1) * 128, :])

    wq_sb = const.tile([128, 4, D], F32)
    wkt_sb = const.tile([128, 4, D], F32)
    wv_sb = const.tile([128, 4, D], F32)
    wo_sb = const.tile([128, 4, D], F32)
    for c in range(4):
        nc.sync.dma_start(out=wq_sb[:, c, :], in_=wq_d[c * 128:(c + 1) * 128, :])
        nc.sync.dma_start(out=wkt_sb[:, c, :], in_=wkt_d[c * 128:(c + 1) * 128, :])
        nc.sync.dma_start(out=wv_sb[:, c, :], in_=wv_d[c * 128:(c + 1) * 128, :])
        nc.sync.dma_start(out=wo_sb[:, c, :], in_=wo_d[c * 128:(c + 1) * 128, :])
    wf1_sb = const.tile([128, 4, DFF], F32)
    for c in range(4):
        nc.sync.dma_start(out=wf1_sb[:, c, :], in_=wf1_d[c * 128:(c + 1) * 128, :])
    wf2_sb = const.tile([128, 16, D], F32)
    for c in range(16):
        nc.sync.dma_start(out=wf2_sb[:, c, :], in_=wf2_d[c * 128:(c + 1) * 128, :])

    ident = const.tile([128, 128], F32)
    make_identity(nc, ident)
    ones = const.tile([1, 512], F32)
    nc.vector.memset(ones, 1.0)
    eps_t = const.tile([128, 1], F32)
    nc.vector.memset(eps_t, EPS)

    def ln_stats(dst_mrs, src, p):
        """dst_mrs: (p,2) tile receiving [mean, rstd]."""
        st = small.tile([128, 6], F32, tag="lnst")
        nc.vector.bn_stats(out=st[:p, :], in_=src)
        mv = small.tile([128, 2], F32, tag="lnmv")
        nc.vector.bn_aggr(out=mv[:p, :], in_=st[:p, :])
        # rstd = 1/sqrt(var+eps)
        nc.scalar.activation(out=dst_mrs[:p, 1:2], in_=mv[:p, 1:2],
                             func=mybir.ActivationFunctionType.Sqrt,
                             bias=eps_t[:p], scale=1.0)
        nc.vector.reciprocal(out=dst_mrs[:p, 1:2], in_=dst_mrs[:p, 1:2])
        nc.vector.tensor_copy(out=dst_mrs[:p, 0:1], in_=mv[:p, 0:1])

    def transpose_to(dst_sb, src_sb, p, f):
        """PE-transpose (p,f) SBUF -> (f,p) SBUF via PSUM. f<=128, p<=128."""
        pt = tp_ps.tile([128, 128], F32, tag="tp")
        nc.tensor.transpose(pt[:f, :p], src_sb, ident[:p, :p])
        nc.vector.tensor_copy(out=dst_sb, in_=pt[:f, :p])

    # ---- stage 1: h~_cls, h_cls, q0, A', kappa ----
    mrs_c = small.tile([1, 2], F32, tag="mrsc")
    ln_stats(mrs_c, V1[0:1, R_CLS, :], 1)
    hcn = const.tile([1, 512], F32)            # h~_cls (normalized, no g/b)
    nc.vector.tensor_scalar(out=hcn, in0=V1[0:1, R_CLS, :],
                            scalar1=mrs_c[0:1, 0:1], scalar2=mrs_c[0:1, 1:2],
                            op0=mybir.AluOpType.subtract,
                            op1=mybir.AluOpType.mult)
    hcls = const.tile([1, 512], F32)           # h_cls = h~_cls*g1 + b1
    nc.vector.tensor_mul(out=hcls, in0=hcn, in1=V1[0:1, R_G1, :])
    nc.vector.tensor_add(out=hcls, in0=hcls, in1=V1[0:1, R_B1, :])

    hcnT = const.tile([128, 4], F32)           # h~_cls transposed (d-major)
    hclsT = const.tile([128, 4], F32)
    g1T = const.tile([128, 4], F32)
    b1T = const.tile([128, 4], F32)
    for c in range(4):
        sl = slice(c * 128, (c + 1) * 128)
        transpose_to(hcnT[:, c:c + 1], hcn[0:1, sl], 1, 128)
        transpose_to(hclsT[:, c:c + 1], hcls[0:1, sl], 1, 128)
        transpose_to(g1T[:, c:c + 1], V1[0:1, R_G1, sl], 1, 128)
        transpose_to(b1T[:, c:c + 1], V1[0:1, R_B1, sl], 1, 128)

    # q0 = h_cls @ Wq + bq   (1,512)
    q0_ps = mm_ps.tile([1, 512], F32, tag="mm")
    for c in range(4):
        nc.tensor.matmul(q0_ps, hclsT[:, c:c + 1], wq_sb[:, c, :],
                         start=(c == 0), stop=(c == 3))
    q0 = const.tile([1, 512], F32)
    nc.vector.tensor_add(out=q0, in0=q0_ps, in1=V1[0:1, R_BQ, :])

    # Q0 block-diagonal (512,8): Q0[d,h] = q0[d] iff d//64==h
    q0T = const.tile([128, 4], F32)
    for c in range(4):
        transpose_to(q0T[:, c:c + 1], q0[0:1, c * 128:(c + 1) * 128], 1, 128)
    Q0 = const.tile([128, 4, H], F32)
    nc.vector.memset(Q0, 0.0)
    for c in range(4):
        nc.vector.tensor_copy(out=Q0[0:64, c, 2 * c:2 * c + 1],
                              in_=q0T[0:64, c:c + 1])
        nc.vector.tensor_copy(out=Q0[64:128, c, 2 * c + 1:2 * c + 2],
                              in_=q0T[64:128, c:c + 1])

    # A[d,h] = sum_{d'} WkT[d',d] Q0[d',h]; then A' = A * g1/8 (per-partition)
    A_ps = mm_ps.tile([128, 4, H], F32, tag="mm")
    for t in range(4):
        for c in range(4):
            nc.tensor.matmul(A_ps[:, t, :],
                             wkt_sb[:, c, t * 128:(t + 1) * 128],
                             Q0[:, c, :], start=(c == 0), stop=(c == 3))
    A_raw = const.tile([128, 4, H], F32)
    nc.vector.tensor_copy(out=A_raw, in_=A_ps)
    Ap = const.tile([128, 4, H], F32)
    for t in range(4):
        nc.vector.tensor_scalar_mul(out=Ap[:, t, :], in0=A_raw[:, t, :],
                                    scalar1=g1T[:, t:t + 1])
    nc.scalar.mul(out=Ap, in_=Ap, mul=1.0 / 8.0)

    # kappa[h] = (b1 @ A[:, h] + bk_blocks . q0_blocks) / 8  -> lhsT row (1,8)
    k2_ps = mm_ps.tile([H, 1], F32, tag="mm")
    for c in range(4):
        nc.tensor.matmul(k2_ps, A_raw[:, c, :], b1T[:, c:c + 1],
                         start=(c == 0), stop=(c == 3))
    bkq = small.tile([1, 512], F32, tag="bkq")
    nc.vector.tensor_mul(out=bkq, in0=V1[0:1, R_BK, :], in1=q0)
    kap1 = small.tile([1, H], F32, tag="kap1")
    nc.vector.reduce_sum(out=kap1, in_=bkq.rearrange("p (h k) -> p h k", h=H),
                         axis=mybir.AxisListType.X)
    k2col = small.tile([H, 1], F32, tag="k2col")
    nc.vector.tensor_copy(out=k2col, in_=k2_ps)
    k2row = small.tile([1, H], F32, tag="k2row")
    transpose_to(k2row, k2col, H, 1)
    kapT = const.tile([1, H], F32)
    nc.vector.tensor_add(out=kapT, in0=k2row, in1=kap1)
    nc.scalar.mul(out=kapT, in_=kapT, mul=1.0 / 8.0)

    # ---- stage 2: LN over x rows (s-major), then transpose h~ to d-major ----
    mrs_x = small.tile([128, 8, 2], F32, tag="mrsx")
    for c in range(8):
        ln_stats(mrs_x[:, c, :], hx[:, c, :], 128)
    for c in range(8):
        nc.vector.tensor_scalar(out=hx[:, c, :], in0=hx[:, c, :],
                                scalar1=mrs_x[:, c, 0:1],
                                scalar2=mrs_x[:, c, 1:2],
                                op0=mybir.AluOpType.subtract,
                                op1=mybir.AluOpType.mult)
    hnT = const.tile([128, 4, ROWS], F32)      # h~ transposed: [d%128, d//128, s]
    for st in range(8):
        for c in range(4):
            pt = tp_ps.tile([128, 128], F32, tag="tp")
            nc.tensor.transpose(pt, hx[:, st, c * 128:(c + 1) * 128],
                                ident)
            nc.vector.tensor_copy(
                out=hnT[:, c, st * 128:(st + 1) * 128], in_=pt)

    # ---- stage 3: per-batch scores -> softmax -> p0 -> uT ----
    uT = const.tile([128, 4, BPC, H], F32)     # [d%128, d//128, b, h] (g folded)
    spT_all = const.tile([1, BPC, H], F32)     # sum_s p0 rows
    for b in range(BPC):
        s_ps = sc_ps.tile([H, S], F32, tag="scores")
        # mask rank-1: ones(8) x maskrow
        nc.tensor.matmul(s_ps, ones[0:1, 0:H], maskr[0:1, b, 0:S],
                         start=True, stop=False, skip_group_check=True)
        # kappa rank-1: kapT x ones(257)
        nc.tensor.matmul(s_ps, kapT, ones[0:1, 0:S],
                         start=False, stop=False, skip_group_check=True)
        # cls column (s=0): A'^T @ h~_clsT
        for c in range(4):
            nc.tensor.matmul(s_ps[:, 0:1], Ap[:, c, :], hcnT[:, c:c + 1],
                             start=False, stop=False, skip_group_check=True)
        # x columns (s=1..256)
        for c in range(4):
            nc.tensor.matmul(s_ps[:, 1:S], Ap[:, c, :],
                             hnT[:, c, b * 256:(b + 1) * 256],
                             start=False, stop=(c == 3), skip_group_check=True)
        # softmax over s, then * bias0
        mx = small.tile([H, 1], F32, tag="mx")
        nc.vector.tensor_reduce(out=mx, in_=s_ps, axis=mybir.AxisListType.X,
                                op=mybir.AluOpType.max, negate=True)
        p0 = work.tile([H, S], F32, tag="p0")
        sume = small.tile([H, 1], F32, tag="sume")
        nc.scalar.activation(out=p0, in_=s_ps,
                             func=mybir.ActivationFunctionType.Exp,
                             bias=mx, scale=1.0, accum_out=sume)
        rec = small.tile([H, 1], F32, tag="rec")
        nc.vector.reciprocal(out=rec, in_=sume)
        nc.vector.tensor_scalar_mul(out=p0, in0=p0, scalar1=rec)
        nc.vector.tensor_mul(out=p0, in0=p0,
                             in1=BIAS0[:, 0:S])
        # sum_s p0 (after bias) and transposes
        sp = small.tile([H, 1], F32, tag="sp")
        nc.vector.reduce_sum(out=sp, in_=p0, axis=mybir.AxisListType.X)
        transpose_to(spT_all[0:1, b, :], sp, H, 1)
        p0T_1 = work.tile([128, H], F32, tag="p0T1")
        transpose_to(p0T_1, p0[:, 1:129], H, 128)
        p0T_2 = work.tile([128, H], F32, tag="p0T2")
        transpose_to(p0T_2, p0[:, 129:257], H, 128)
        p0Tc = work.tile([1, H], F32, tag="p0Tc")
        transpose_to(p0Tc, p0[:, 0:1], H, 1)
        # uT[d, b, h] = sum_s h~[s,d] p0[h,s]  (+ b1 rank-1 via spT)
        for t in range(4):
            u_ps = mm_ps.tile([128, H], F32, tag="mm")
            nc.tensor.matmul(u_ps, hx_dslice(hx, 2 * b, t), p0T_1,
                             start=True, stop=False)
            nc.tensor.matmul(u_ps, hx_dslice(hx, 2 * b + 1, t), p0T_2,
                             start=False, stop=False)
            nc.tensor.matmul(u_ps, hcn[0:1, t * 128:(t + 1) * 128], p0Tc,
                             start=False, stop=False)
            nc.tensor.matmul(u_ps, V1[0:1, R_B1, t * 128:(t + 1) * 128],
                             spT_all[0:1, b, :], start=False, stop=True)
            # fold g1 (per-partition d scaling)
            nc.vector.tensor_scalar_mul(out=uT[:, t, b, :], in0=u_ps,
                                        scalar1=g1T[:, t:t + 1])

    # ---- stage 4: o0 = per-head u @ Wv + sp*bv; h1 = h_cls + o0@Wo + bo ----
    o0_ps = mm_ps.tile([BPC, D], F32, tag="mm")
    for h in range(H):
        cols = slice(h * DK, (h + 1) * DK)
        for t in range(4):
            nc.tensor.matmul(o0_ps[:, cols], uT[:, t, :, h], wv_sb[:, t, cols],
                             start=(t == 0), stop=False)
        nc.tensor.matmul(o0_ps[:, cols], spT_all[0:1, :, h],
                         V1[0:1, R_BV, cols], start=False, stop=True)
    o0 = work.tile([BPC, D], F32, tag="o0sb")
    nc.vector.tensor_copy(out=o0, in_=o0_ps)
    o0T = work.tile([128, 4, BPC], F32, tag="o0T")
    for c in range(4):
        transpose_to(o0T[:, c, :], o0[:, c * 128:(c + 1) * 128], BPC, 128)
    hclsbo = small.tile([1, 512], F32, tag="hclsbo")
    nc.vector.tensor_add(out=hclsbo, in0=hcls, in1=V1[0:1, R_BO, :])
    h1_ps = mm_ps.tile([BPC, D], F32, tag="mm")
    for c in range(4):
        nc.tensor.matmul(h1_ps, o0T[:, c, :], wo_sb[:, c, :],
                         start=(c == 0), stop=False)
    nc.tensor.matmul(h1_ps, ones[0:1, 0:BPC], hclsbo, start=False, stop=True)
    h1 = work.tile([BPC, D], F32, tag="h1sb")
    nc.vector.tensor_copy(out=h1, in_=h1_ps)

    # ---- stage 5: LN2 -> g2 (with g/b via broadcast rows) -> g2T ----
    mrs2 = small.tile([BPC, 2], F32, tag="mrs2")
    ln_stats(mrs2, h1, BPC)
    g2 = work.tile([BPC, D], F32, tag="g2")
    nc.vector.tensor_scalar(out=g2, in0=h1, scalar1=mrs2[:, 0:1],
                            scalar2=mrs2[:, 1:2],
                            op0=mybir.AluOpType.subtract,
                            op1=mybir.AluOpType.mult)
    g2row = work.tile([BPC, 512], F32, tag="brow")
    b2row = work.tile([BPC, 512], F32, tag="brow2")
    nc.sync.dma_start(out=g2row, in_=vecs_d[R_G2:R_G2 + 1, :].to_broadcast((BPC, 512)))
    nc.sync.dma_start(out=b2row, in_=vecs_d[R_B2:R_B2 + 1, :].to_broadcast((BPC, 512)))
    nc.vector.tensor_mul(out=g2, in0=g2, in1=g2row)
    nc.vector.tensor_add(out=g2, in0=g2, in1=b2row)
    g2T = work.tile([128, 4, BPC], F32, tag="g2T")
    for c in range(4):
        transpose_to(g2T[:, c, :], g2[:, c * 128:(c + 1) * 128], BPC, 128)

    # bf1 as per-partition columns: bf1T[p, cc, r] for f = r*512 + cc*128 + p
    bf1T = const.tile([128, 4, 4], F32)
    for cc in range(4):
        transpose_to(bf1T[:, cc, :], BF1[0:4, cc * 128:(cc + 1) * 128],
                     4, 128)

    # ---- stage 6: FFN, computed transposed: a1T[f,b] ----
    gel = work.tile([128, 16, BPC], F32, tag="gel")
    for fc in range(16):
        a1_ps = ff_ps.tile([128, BPC], F32, tag="a1")
        for c in range(4):
            nc.tensor.matmul(a1_ps, wf1_sb[:, c, fc * 128:(fc + 1) * 128],
                             g2T[:, c, :], start=(c == 0), stop=(c == 3))
        nc.scalar.activation(out=gel[:, fc, :], in_=a1_ps,
                             func=mybir.ActivationFunctionType.Gelu,
                             bias=bf1T[:, fc % 4, fc // 4:fc // 4 + 1],
                             scale=1.0)
    f2_ps = ff_ps.tile([BPC, D], F32, tag="f2")
    for fc in range(16):
        nc.tensor.matmul(f2_ps, gel[:, fc, :], wf2_sb[:, fc, :],
                         start=(fc == 0), stop=False)
    nc.tensor.matmul(f2_ps, ones[0:1, 0:BPC], V1[0:1, R_BF2, :],
                     start=False, stop=True)

    # ---- stage 7: final LN and output ----
    lo = work.tile([BPC, D], F32, tag="lo")
    nc.vector.tensor_add(out=lo, in0=h1, in1=f2_ps)
    mrsf = small.tile([BPC, 2], F32, tag="mrsf")
    ln_stats(mrsf, lo, BPC)
    nc.vector.tensor_scalar(out=lo, in0=lo, scalar1=mrsf[:, 0:1],
                            scalar2=mrsf[:, 1:2],
                            op0=mybir.AluOpType.subtract,
                            op1=mybir.AluOpType.mult)
    gfrow = work.tile([BPC, 512], F32, tag="brow")
    bfrow = work.tile([BPC, 512], F32, tag="brow2")
    nc.sync.dma_start(out=gfrow, in_=vecs_d[R_GF:R_GF + 1, :].to_broadcast((BPC, 512)))
    nc.sync.dma_start(out=bfrow, in_=vecs_d[R_BF:R_BF + 1, :].to_broadcast((BPC, 512)))
    nc.vector.tensor_mul(out=lo, in0=lo, in1=gfrow)
    nc.vector.tensor_add(out=lo, in0=lo, in1=bfrow)
    nc.sync.dma_start(out=out_d[:, :], in_=lo)


def hx_dslice(hn, chunk, t):
    return hn[:, chunk, t * 128:(t + 1) * 128]


def _prep_host(inputs):
    l = LAYER
    f = lambda k: np.ascontiguousarray(np.asarray(inputs[k], np.float32))
    x = f("x")                                   # (32,256,512)
    Wq = f("Wq")[l]
    WkT = np.ascontiguousarray(f("Wk")[l].T)
    Wv = f("Wv")[l]
    Wo = f("Wo")[l]
    Wf1 = f("Wf1")[l]
    Wf2 = f("Wf2")[l]
    cm = np.asarray(inputs["channel_mask"])      # (32,256) bool

    vecs = np.zeros((NV, 512), np.float32)
    vecs[R_CLS] = f("cls_token").reshape(-1)
    vecs[R_G1] = f("ln1_g")[l]; vecs[R_B1] = f("ln1_b")[l]
    vecs[R_BQ] = f("bq")[l]; vecs[R_BK] = f("bk")[l]
    vecs[R_BV] = f("bv")[l]; vecs[R_BO] = f("bo")[l]
    vecs[R_G2] = f("ln2_g")[l]; vecs[R_B2] = f("ln2_b")[l]
    vecs[R_GF] = f("lnf_g"); vecs[R_BF] = f("lnf_b")
    vecs[R_BF2] = f("bf2")[l]
    vecs[R_BF1:R_BF1 + 4] = f("bf1")[l].reshape(4, 512)
    vecs[R_BIAS0:R_BIAS0 + H, :S] = f("attn_bias")[:, 0, :]

    # additive key mask: key 0 (CLS) always blocked; x keys blocked where
    # channel_mask is False  (key_block = ~concat([0, mask]))
    maskadd = np.zeros((B, S), np.float32)
    maskadd[:, 0] = NEG
    maskadd[:, 1:][~cm] = NEG

    in_maps = []
    for core in range(NCORES):
        bs = slice(core * BPC, (core + 1) * BPC)
        v = vecs.copy()
        v[R_MASK:R_MASK + BPC, :S] = maskadd[bs]
        in_maps.append(dict(
            x=np.ascontiguousarray(x[bs].reshape(ROWS, D)),
            wq=Wq, wkt=WkT, wv=Wv, wo=Wo, wf1=Wf1, wf2=Wf2,
            vecs=v,
        ))
    return in_maps


def kernel(**inputs):
    global LAST_RESULTS
    if "nc" not in _CACHE:
        _CACHE["nc"] = _build_nc()
    nc = _CACHE["nc"]
    in_maps = _prep_host(inputs)
    res = run_bass_kernel_spmd(nc, in_maps, list(range(NCORES)))
    LAST_RESULTS = res
    out = np.concatenate([res.results[i]["out"] for i in range(NCORES)], axis=0)
    return out.astype(np.float32)


if __name__ == "__main__":
    nc = _build_nc()
    print("build OK")
